# revision 17
# baseline (speedup 1.0000x reference)
"""Bass/Trainium2 kernel for CausalSelfAttention (B=8, T=1024, C=768, H=12).

Sharding: data-parallel over batch. 8 cores, one batch element per core.
No collectives. Each core runs an identical SPMD program on its own slice.

v2.2: Q/K projection via fp8-e4m3 DoubleRow matmuls (2x PE throughput) with a
residual decomposition of x (x ~= x8 + xr, both fp8, two accumulating groups)
so only W_qk's fp8 quantization (~1% on the output) touches accuracy.
V / S / PV / output projection in bf16.  Attention inner loop decoupled from
the ACT exp cadence by a 5-deep shared psum pool; softmax normalization
(copy sums -> DMA to partition 0 -> partition_broadcast -> reciprocal ->
multiply) is emitted piecewise into the NEXT unit's kt loop so its cross-
engine latency never head-of-line-blocks DVE or stalls PE; independent
projection work is interleaved as fillers to keep PE at full p-state.

Per-core layouts (host-prepared; WS=64 fp8 weight prescale):
  xT    [128, 6, 1024] bf16   x[b].T   (row c = k*128 + p)
  x8    [128, 3, 2, 1024] f8e4  fp8(x.T) in DoubleRow layout c=kt*256+i*128+p
  xr    [128, 3, 2, 1024] f8e4  fp8(x.T - x8)
  wqk8  [128, 12, 3, 2, 128] f8e4  (W_attn[:, :1536] * WS, Q cols /8), m-major
  wv    [128, 6, 768]  bf16   W_attn[:, 1536:]
  wp    [128, 6, 768]  bf16   W_proj
  bqk   [128, 12]  f32   b_attn[:1536] per-m-tile columns (Q part /8)
  bv    [128, 768] f32   b_attn[1536:] broadcast over partitions
  bp    [128, 768] f32   b_proj broadcast over partitions
  qm    [128, 8]   f32   query_mask as per-partition columns per q-tile
  dm    [128, 128] bf16  causal diagonal-block mask, [key, query] orientation
Output: y [1024, 768] f32 per core.
"""

import sys

if "/opt/trn_rl_repo" not in sys.path:
    sys.path.insert(0, "/opt/trn_rl_repo")

import numpy as np
import ml_dtypes

import concourse.bass as bass
import concourse.bacc as bacc
import concourse.mybir as mybir
import concourse.tile as tile
from concourse.bass import ts, ds

BF16 = mybir.dt.bfloat16
F8 = mybir.dt.float8e4
F32 = mybir.dt.float32
AF = mybir.ActivationFunctionType
ALU = mybir.AluOpType
DR = mybir.MatmulPerfMode.DoubleRow
BF16NP = ml_dtypes.bfloat16
F8NP = ml_dtypes.float8_e4m3

T, C, H, HD = 1024, 768, 12, 64
NCORES = 8
WS = 64.0  # fp8 weight prescale (keeps W ~ N(0,0.02) out of f8 subnormals)

_CACHE = {}


def build_program():
    nc = bacc.Bacc("TRN2", target_bir_lowering=False, debug=False)

    xT_d = nc.dram_tensor("xT", [128, 6, T], BF16, kind="ExternalInput")
    x8_d = nc.dram_tensor("x8", [128, 3, 2, T], F8, kind="ExternalInput")
    xr_d = nc.dram_tensor("xr", [128, 3, 2, T], F8, kind="ExternalInput")
    wqk8_d = nc.dram_tensor("wqk8", [128, 12, 3, 2, 128], F8, kind="ExternalInput")
    wv_d = nc.dram_tensor("wv", [128, 6, C], BF16, kind="ExternalInput")
    wp_d = nc.dram_tensor("wp", [128, 6, C], BF16, kind="ExternalInput")
    bqk_d = nc.dram_tensor("bqk", [128, 12], F32, kind="ExternalInput")
    bv_d = nc.dram_tensor("bv", [128, C], F32, kind="ExternalInput")
    bp_d = nc.dram_tensor("bp", [128, C], F32, kind="ExternalInput")
    qm_d = nc.dram_tensor("qm", [128, 8], F32, kind="ExternalInput")
    dm_d = nc.dram_tensor("dm", [128, 128], BF16, kind="ExternalInput")
    y_d = nc.dram_tensor("y", [T, C], F32, kind="ExternalOutput")

    with tile.TileContext(nc) as tc:
        with (
            tc.tile_pool(name="const", bufs=1) as cp,
            tc.tile_pool(name="ptp", bufs=10) as ptp,
            tc.tile_pool(name="rsp", bufs=2) as rsp,
            tc.tile_pool(name="rmp", bufs=2) as rmp,
            tc.tile_pool(name="bcp", bufs=2) as bcp,
            tc.tile_pool(name="rcp", bufs=2) as rcp,
            tc.tile_pool(name="otxp", bufs=2) as otxp,
            tc.tile_pool(name="ysb", bufs=2) as ysbp,
            tc.tile_pool(name="ps_s", bufs=3, space="PSUM") as ps_s,
            tc.tile_pool(name="ps_f", bufs=2, space="PSUM") as psp,
            tc.tile_pool(name="ps_o", bufs=3, space="PSUM") as ps_o,
        ):
            # ---------------- persistent SBUF tensors ----------------
            xT_sb = cp.tile([128, 6, T], BF16, name="xT_sb")
            x8_sb = cp.tile([128, 3, 2, T], F8, name="x8_sb")
            xr_sb = cp.tile([128, 3, 2, T], F8, name="xr_sb")
            wqk_sb = cp.tile([128, 12, 3, 2, 128], F8, name="wqk_sb")
            wv_sb = cp.tile([128, 6, C], BF16, name="wv_sb")
            wp_sb = cp.tile([128, 6, C], BF16, name="wp_sb")
            bqk_sb = cp.tile([128, 12], F32, name="bqk_sb")
            bv_sb = cp.tile([128, C], F32, name="bv_sb")
            bp_sb = cp.tile([128, C], F32, name="bp_sb")
            qm_sb = cp.tile([128, 8], F32, name="qm_sb")
            dm_sb = cp.tile([128, 128], BF16, name="dm_sb")
            qk_sb = [cp.tile([128, T], BF16, name=f"qk{m}") for m in range(12)]
            v_sb = [cp.tile([128, 12 * 65], BF16, name=f"v{t}") for t in range(8)]
            ot_sb = cp.tile([128, 6, T], BF16, name="ot_sb")

            # ---------------- loads ----------------
            # sync queue: the Q/K projection path (needed first)
            nc.sync.dma_start(bqk_sb[:], bqk_d[:, :])
            nc.sync.dma_start(x8_sb[:], x8_d[:, :, :, :])
            nc.sync.dma_start(wqk_sb[:, 0], wqk8_d[:, 0])
            nc.sync.dma_start(wqk_sb[:, 6], wqk8_d[:, 6])
            nc.sync.dma_start(wqk_sb[:, 1], wqk8_d[:, 1])
            nc.sync.dma_start(wqk_sb[:, 7], wqk8_d[:, 7])
            nc.sync.dma_start(wqk_sb[:, 2:6], wqk8_d[:, 2:6])
            nc.sync.dma_start(wqk_sb[:, 8:12], wqk8_d[:, 8:12])
            nc.sync.dma_start(bp_sb[:], bp_d[:, :])
            nc.sync.dma_start(wp_sb[:, :, 0:384], wp_d[:, :, 0:384])
            nc.sync.dma_start(wp_sb[:, :, 384:768], wp_d[:, :, 384:768])
            # gpsimd queue: xr (2nd accumulation group), then the V path
            nc.gpsimd.dma_start(xr_sb[:], xr_d[:, :, :, :])
            nc.gpsimd.dma_start(bv_sb[:], bv_d[:, :])
            nc.gpsimd.dma_start(wv_sb[:, :, 0:384], wv_d[:, :, 0:384])
            nc.gpsimd.dma_start(wv_sb[:, :, 384:768], wv_d[:, :, 384:768])
            nc.gpsimd.dma_start(xT_sb[:, :, 0:512], xT_d[:, :, 0:512])
            nc.gpsimd.dma_start(xT_sb[:, :, 512:1024], xT_d[:, :, 512:1024])
            nc.gpsimd.dma_start(qm_sb[:], qm_d[:, :])
            nc.gpsimd.dma_start(dm_sb[:], dm_d[:, :])
            # ones columns interleaved into V (produce softmax sums during PV)
            for t in range(8):
                nc.gpsimd.memset(
                    v_sb[t].rearrange("p (h d) -> p h d", d=65)[:, :, 64:65],
                    1.0,
                )

            # ---------------- emit helpers ----------------
            def emit_qk(m):
                # qk_sb[m] = x @ W_qk[:, m-tile] + b
                # fp8 DoubleRow, two accumulating groups (x8 then xr)
                for j in range(2):
                    ps = psp.tile([128, 512], F32, name="psqk", tag="pp")
                    for gi, xg in enumerate((x8_sb, xr_sb)):
                        for kt in range(3):
                            nc.tensor.matmul(
                                ps[:],
                                wqk_sb[:, m, kt],
                                xg[:, kt, :, ts(j, 512)],
                                start=(gi == 0 and kt == 0),
                                stop=(gi == 1 and kt == 2),
                                perf_mode=DR,
                            )
                    nc.vector.tensor_scalar(
                        out=qk_sb[m][:, ts(j, 512)],
                        in0=ps[:],
                        scalar1=1.0 / WS,
                        scalar2=bqk_sb[:, m : m + 1],
                        op0=ALU.mult,
                        op1=ALU.add,
                    )

            def emit_v(t):
                # v_sb[t] = x[t-tile] @ W_v + bv   (bf16)
                for c0, cw in ((0, 512), (512, 256)):
                    ps = psp.tile([128, 512], F32, name="psv", tag="pp")
                    for k in range(6):
                        nc.tensor.matmul(
                            ps[:, :cw],
                            xT_sb[:, k, ts(t, 128)],
                            wv_sb[:, k, ds(c0, cw)],
                            start=(k == 0),
                            stop=(k == 5),
                        )
                    nh, h0 = cw // 64, c0 // 64
                    nc.vector.tensor_add(
                        v_sb[t].rearrange("p (h d) -> p h d", d=65)[
                            :, h0 : h0 + nh, 0:64
                        ],
                        ps[:, :cw].rearrange("p (h d) -> p h d", d=64),
                        bv_sb[:, ds(c0, cw)].rearrange("p (h d) -> p h d", d=64),
                    )

            def emit_e(qt):
                # y[qt-tile] = (OT.T @ W_proj) * qm + bp   (bf16)
                ysb = ysbp.tile([128, C], F32, name="ysb", tag="ysb")
                for c0, cw in ((0, 512), (512, 256)):
                    ps = psp.tile([128, 512], F32, name="psy", tag="pp")
                    for k in range(6):
                        nc.tensor.matmul(
                            ps[:, :cw],
                            ot_sb[:, k, ts(qt, 128)],
                            wp_sb[:, k, ds(c0, cw)],
                            start=(k == 0),
                            stop=(k == 5),
                        )
                    nc.vector.scalar_tensor_tensor(
                        out=ysb[:, ds(c0, cw)],
                        in0=ps[:, :cw],
                        scalar=qm_sb[:, qt : qt + 1],
                        in1=bp_sb[:, ds(c0, cw)],
                        op0=ALU.mult,
                        op1=ALU.add,
                    )
                nc.sync.dma_start(y_d[ts(qt, 128), :], ysb[:])

            def att_head(p, sbi):
                # emit S/exp/dm for the first two processed key-tiles of the
                # unit; PVs are delayed so this can overlap the previous
                # unit's tail.
                q0 = sbi * 512
                nkt = 4 + 4 * sbi
                # process masked (diagonal) key-tiles first so the unit tail
                # has no Pool hop in its S->exp->PV chain
                kt_order = (
                    list(range(nkt)) if sbi == 0 else [4, 5, 6, 7, 0, 1, 2, 3]
                )
                st = {
                    "p": p,
                    "sbi": sbi,
                    "q0": q0,
                    "nkt": nkt,
                    "kt_order": kt_order,
                    "ptts": {},
                    "psO": [
                        ps_o.tile([128, 512], F32, name=f"psO{hh}", tag="o")
                        for hh in range(2)
                    ],
                }

                def ablock(ki):
                    kt = kt_order[ki]
                    dc = max(0, kt * 128 - q0)
                    w = 512 - dc
                    for hh in range(2):
                        psS = ps_s.tile([128, 512], F32, name="psS", tag="ps")
                        nc.tensor.matmul(
                            psS[:, ds(dc, w)],
                            qk_sb[6 + p][ds(64 * hh, 64), ts(kt, 128)],
                            qk_sb[p][ds(64 * hh, 64), ds(q0 + dc, w)],
                            start=True,
                            stop=True,
                        )
                        ptt = ptp.tile([128, 512], BF16, name="ptt", tag="ptt")
                        nc.scalar.activation(
                            ptt[:, ds(dc, w)], psS[:, ds(dc, w)], AF.Exp
                        )
                        if kt * 128 >= q0:
                            nc.gpsimd.tensor_mul(
                                ptt[:, ds(dc, 128)],
                                ptt[:, ds(dc, 128)],
                                dm_sb[:, :],
                            )
                        st["ptts"][(kt, hh)] = ptt

                def pv(ki):
                    kt = kt_order[ki]
                    dc = max(0, kt * 128 - q0)
                    w = 512 - dc
                    for hh in range(2):
                        h = 2 * p + hh
                        nc.tensor.matmul(
                            st["psO"][hh][0:65, ds(dc, w)],
                            v_sb[kt][:, h * 65 : h * 65 + 65],
                            st["ptts"][(kt, hh)][:, ds(dc, w)],
                            start=(ki == 0),
                            stop=(ki == nkt - 1),
                            skip_group_check=True,
                        )

                st["ablock"] = ablock
                st["pv"] = pv
                ablock(0)
                ablock(1)
                return st

            def att_rest(st, fills, chain, late_fills=False):
                # emit the remaining A-blocks with PVs delayed by two blocks,
                # the previous unit's normalization chain pieces, and fillers.
                # Returns (tail, chain): `tail` emits the last two PVs and the
                # normalization head and must run after the NEXT unit's
                # att_head; `chain` pieces run inside the next unit's rest.
                p, sbi, q0, nkt = st["p"], st["sbi"], st["q0"], st["nkt"]
                ablock, pv, psO = st["ablock"], st["pv"], st["psO"]
                fi = 0
                if late_fills:
                    fill_slots = (7,)
                elif sbi == 0:
                    fill_slots = (3,)
                else:
                    fill_slots = (3, 5, 7)

                def fill():
                    nonlocal fi
                    if fi < len(fills):
                        fills[fi]()
                        fi += 1

                mule_ki = 5 if sbi == 1 else None
                for ki in range(2, nkt):
                    ablock(ki)
                    if chain and ki == 2:
                        chain.pop("pb")()
                    if chain and ki == 3:
                        chain.pop("recip")()
                    if chain and ki == mule_ki:
                        chain.pop("muls")()
                    pv(ki - 2)
                    if ki in fill_slots:
                        fill()
                if chain and "muls" in chain:
                    chain.pop("muls")()
                while fi < len(fills):
                    fill()

                rs = rsp.tile([65, 2, 512], F32, name="rs", tag="rs")
                rm = rmp.tile([1, 2, 512], F32, name="rm", tag="rm")
                bcs = bcp.tile([64, 2, 512], F32, name="bcs", tag="bcs")
                rc = rcp.tile([64, 2, 512], F32, name="rc", tag="rc")

                def tail():
                    pv(nkt - 2)
                    pv(nkt - 1)
                    # normalization head: copy sums (psum row 64) to SBUF and
                    # DMA them to partition 0; rest deferred to the next unit.
                    for hh in range(2):
                        nc.vector.tensor_copy(
                            rs[64:65, hh, :], psO[hh][64:65, :]
                        )
                    nc.sync.dma_start(rm[0:1], rs[64:65])

                def piece_pb():
                    for hh in range(2):
                        nc.gpsimd.partition_broadcast(
                            bcs[:, hh, :], rm[0:1, hh, :]
                        )

                def piece_recip():
                    nc.vector.reciprocal_approx_fast(rc[:], bcs[:])

                def piece_muls():
                    nc.vector.tensor_mul(
                        ot_sb[0:64, p, ds(q0, 512)],
                        psO[0][0:64, :],
                        rc[:, 0, :],
                    )
                    otx = otxp.tile([64, 512], BF16, name="otx", tag="otx")
                    nc.vector.tensor_mul(otx[:], psO[1][0:64, :], rc[:, 1, :])
                    nc.sync.dma_start(ot_sb[64:128, p, ds(q0, 512)], otx[:])

                return tail, {
                    "pb": piece_pb,
                    "recip": piece_recip,
                    "muls": piece_muls,
                }

            # ---------------- main schedule ----------------
            for m in (0, 6, 1, 7):
                emit_qk(m)
            for t in range(4):
                emit_v(t)
            fills_by_unit = {
                (0, 0): [lambda t=t: emit_v(t) for t in range(4, 8)],
                (0, 1): [lambda: emit_qk(2), lambda: emit_qk(8)],
                (1, 0): [lambda: emit_qk(3)],
                (1, 1): [lambda: emit_qk(9)],
                (2, 0): [lambda: emit_qk(4)],
                (2, 1): [lambda: emit_qk(10)],
                (3, 0): [lambda: emit_qk(5)],
                (3, 1): [lambda: emit_qk(11)],
                (4, 0): [],
                (4, 1): [],
                (5, 0): [],
                (5, 1): [lambda q=q: emit_e(q) for q in range(4)],
            }
            units = [(p, sbi) for p in range(6) for sbi in (0, 1)]
            st = att_head(*units[0])
            chain = None
            for idx, u in enumerate(units):
                tail, chain = att_rest(
                    st, fills_by_unit[u], chain, late_fills=(u == (5, 1))
                )
                if idx + 1 < len(units):
                    st = att_head(*units[idx + 1])
                tail()
            # drain the last unit's chain, then the remaining output tiles
            chain["pb"]()
            chain["recip"]()
            chain["muls"]()
            for qt in range(4, 8):
                emit_e(qt)

    nc.compile()
    return nc


def _get_nc():
    if "nc" not in _CACHE:
        _CACHE["nc"] = build_program()
    return _CACHE["nc"]


def _drpack(M, dtype):
    """[768, N] -> [128, 3, 2, N] DoubleRow layout
    (contraction row c = kt*256 + i*128 + p)."""
    N = M.shape[1]
    return np.ascontiguousarray(
        M.reshape(3, 2, 128, N).transpose(2, 0, 1, 3)
    ).astype(dtype)


def _kpack(M, dtype):
    """[768, N] -> [128, 6, N] (row c = k*128 + p)."""
    N = M.shape[1]
    return np.ascontiguousarray(M.reshape(6, 128, N).transpose(1, 0, 2)).astype(
        dtype
    )


def prep_core_inputs(x, mask, query_mask, W_attn, b_attn, W_proj, b_proj):
    """Host-side prep. Returns a list of per-core input dicts."""
    scale = 1.0 / np.sqrt(HD)
    W = np.asarray(W_attn, np.float32)
    Wqk = W[:, : 2 * C].copy()
    Wqk[:, :C] *= scale
    b_s = np.asarray(b_attn, np.float32).copy()
    b_s[:C] *= scale

    wqk8 = _drpack(Wqk * WS, F8NP)  # [128, 3, 2, 1536]
    # m-major layout [128, 12, 3, 2, 128]
    wqk8 = np.ascontiguousarray(
        wqk8.reshape(128, 3, 2, 12, 128).transpose(0, 3, 1, 2, 4)
    )
    shared = {
        "wqk8": wqk8,
        "wv": _kpack(W[:, 2 * C :], BF16NP),
        "wp": _kpack(np.asarray(W_proj, np.float32), BF16NP),
        "bqk": np.ascontiguousarray(b_s[: 2 * C].reshape(12, 128).T),
        "bv": np.ascontiguousarray(
            np.broadcast_to(b_s[2 * C :], (128, C))
        ).astype(np.float32),
        "bp": np.ascontiguousarray(
            np.broadcast_to(np.asarray(b_proj, np.float32), (128, C))
        ),
        # causal diagonal-block mask (identical for every block): [key, query]
        "dm": np.triu(np.ones((128, 128), np.float32)).astype(BF16NP),
    }

    per_core = []
    for b in range(NCORES):
        xT = np.asarray(x[b], np.float32).T  # [768, 1024]
        x8 = _drpack(xT, F8NP)
        # residual: xr = fp8(xT - fp8(xT)) in the same DR layout
        xr = (_drpack(xT, np.float32) - x8.astype(np.float32)).astype(F8NP)
        qm = np.ascontiguousarray(
            np.asarray(query_mask[b, 0, :, 0], np.float32).reshape(8, 128).T
        )
        per_core.append(
            {
                "xT": _kpack(xT, BF16NP),
                "x8": x8,
                "xr": xr,
                "qm": qm,
                **shared,
            }
        )
    return per_core


def run_on_cores(inputs, trace=False, **kw):
    from concourse.bass_utils import run_bass_kernel_spmd

    nc = _get_nc()
    in_maps = prep_core_inputs(**inputs)
    res = run_bass_kernel_spmd(
        nc, in_maps, core_ids=list(range(NCORES)), trace=trace, **kw
    )
    out = np.stack([res.results[b]["y"] for b in range(NCORES)], axis=0)
    return out.astype(np.float32), res


def kernel(**inputs) -> np.ndarray:
    out, _ = run_on_cores(inputs, trace=False)
    return out


# revision 18
# speedup vs baseline: 1.8243x; 1.8243x over previous
"""Bass/Trainium2 kernel for CausalSelfAttention (B=8, T=1024, C=768, H=12).

Sharding: data-parallel over batch. 8 cores, one batch element per core.
No collectives. Each core runs an identical SPMD program on its own slice.

v3 = the v1 structure plus two surgical changes:
 1. Q/K projection via fp8-e4m3 DoubleRow matmuls (half the PE rows) with a
    residual decomposition of x (x ~= x8 + xr, two accumulating DR groups),
    so only W_qk's fp8 quantization (~1% out) touches accuracy.
 2. Each (pair, sbi) unit's softmax-normalization block (PE broadcast matmul
    + reciprocal + multiplies) is emitted one unit LATE, so the PE broadcast
    never waits on the DVE sums-copy and PE keeps its p-state.

Per-core layouts (host-prepared; WS=64 fp8 weight prescale):
  xT    [128, 6, 1024] bf16   x[b].T  (row c = k*128 + p)
  x8    [128, 3, 2, 1024] f8e4  fp8(x.T) in DoubleRow layout c=kt*256+i*128+p
  xr    [128, 3, 2, 1024] f8e4  fp8(x.T - x8)
  wqk8  [128, 12, 3, 2, 128] f8e4  (W_attn[:, :1536] * WS, Q cols /8), m-major
  wv    [128, 6, 768]  bf16   W_attn[:, 1536:]
  wp    [128, 6, 768]  bf16   W_proj
  bqk   [128, 12]  f32   b_attn[:1536] per-m-tile columns (Q part /8)
  bv    [128, 768] f32   b_attn[1536:] broadcast over partitions
  bp    [128, 768] f32   b_proj broadcast over partitions
  qm    [128, 8]   f32   query_mask as per-partition columns per q-tile
  dm    [128, 128] bf16  causal diagonal-block mask, [key, query] orientation
Output: y [1024, 768] f32 per core.
"""

import sys

if "/opt/trn_rl_repo" not in sys.path:
    sys.path.insert(0, "/opt/trn_rl_repo")

import numpy as np
import ml_dtypes

import concourse.bass as bass
import concourse.bacc as bacc
import concourse.mybir as mybir
import concourse.tile as tile
from concourse.bass import ts, ds

BF16 = mybir.dt.bfloat16
F8 = mybir.dt.float8e4
F32 = mybir.dt.float32
AF = mybir.ActivationFunctionType
ALU = mybir.AluOpType
DR = mybir.MatmulPerfMode.DoubleRow
BF16NP = ml_dtypes.bfloat16
F8NP = ml_dtypes.float8_e4m3

T, C, H, HD = 1024, 768, 12, 64
NCORES = 8
WS = 64.0

_CACHE = {}


def build_program():
    """Build the single-core SPMD Bass program."""
    nc = bacc.Bacc("TRN2", target_bir_lowering=False, debug=False)

    xT_d = nc.dram_tensor("xT", [128, 6, T], BF16, kind="ExternalInput")
    x8_d = nc.dram_tensor("x8", [128, 3, 2, T], F8, kind="ExternalInput")
    xr_d = nc.dram_tensor("xr", [128, 3, 2, T], F8, kind="ExternalInput")
    wqk8_d = nc.dram_tensor("wqk8", [128, 12, 3, 2, 128], F8, kind="ExternalInput")
    wv_d = nc.dram_tensor("wv", [128, 6, C], BF16, kind="ExternalInput")
    wp_d = nc.dram_tensor("wp", [128, 6, C], BF16, kind="ExternalInput")
    bqk_d = nc.dram_tensor("bqk", [128, 12], F32, kind="ExternalInput")
    bv_d = nc.dram_tensor("bv", [128, C], F32, kind="ExternalInput")
    bp_d = nc.dram_tensor("bp", [128, C], F32, kind="ExternalInput")
    qm_d = nc.dram_tensor("qm", [128, 8], F32, kind="ExternalInput")
    dm_d = nc.dram_tensor("dm", [128, 128], BF16, kind="ExternalInput")
    y_d = nc.dram_tensor("y", [T, C], F32, kind="ExternalOutput")

    with tile.TileContext(nc) as tc:
        with (
            tc.tile_pool(name="const", bufs=1) as cp,
            tc.tile_pool(name="ptp", bufs=10) as ptp,
            tc.tile_pool(name="recp", bufs=3) as recp,
            tc.tile_pool(name="bcp", bufs=3) as bcp,
            tc.tile_pool(name="otxp", bufs=3) as otxp,
            tc.tile_pool(name="ysb", bufs=3) as ysbp,
            tc.tile_pool(name="ps_a", bufs=4, space="PSUM") as ps_a,
            tc.tile_pool(name="ps_o", bufs=3, space="PSUM") as ps_o,
            tc.tile_pool(name="ps_bc", bufs=1, space="PSUM") as ps_bc,
        ):
            # ---------------- persistent SBUF tensors ----------------
            xT_sb = cp.tile([128, 6, T], BF16, name="xT_sb")
            x8_sb = cp.tile([128, 3, 2, T], F8, name="x8_sb")
            xr_sb = cp.tile([128, 3, 2, T], F8, name="xr_sb")
            wqk_sb = cp.tile([128, 12, 3, 2, 128], F8, name="wqk_sb")
            wv_sb = cp.tile([128, 6, C], BF16, name="wv_sb")
            wp_sb = cp.tile([128, 6, C], BF16, name="wp_sb")
            bqk_sb = cp.tile([128, 12], F32, name="bqk_sb")
            bv_sb = cp.tile([128, C], F32, name="bv_sb")
            bp_sb = cp.tile([128, C], F32, name="bp_sb")
            qm_sb = cp.tile([128, 8], F32, name="qm_sb")
            dm_sb = cp.tile([128, 128], BF16, name="dm_sb")
            ones_sb = cp.tile([128, 64], F32, name="ones_sb")
            onesr_sb = cp.tile([128, 64], mybir.dt.float32r, name="onesr_sb")
            qk_sb = [cp.tile([128, T], BF16, name=f"qk{m}") for m in range(12)]
            v_sb = [cp.tile([128, 12 * 65], BF16, name=f"v{t}") for t in range(8)]
            ot_sb = cp.tile([128, 6, T], BF16, name="ot_sb")

            # ---------------- loads ----------------
            nc.sync.dma_start(bqk_sb[:], bqk_d[:, :])
            nc.sync.dma_start(x8_sb[:], x8_d[:, :, :, :])
            nc.sync.dma_start(wqk_sb[:, 0], wqk8_d[:, 0])
            nc.sync.dma_start(wqk_sb[:, 6], wqk8_d[:, 6])
            nc.sync.dma_start(wqk_sb[:, 1], wqk8_d[:, 1])
            nc.sync.dma_start(wqk_sb[:, 7], wqk8_d[:, 7])
            nc.sync.dma_start(wqk_sb[:, 2:6], wqk8_d[:, 2:6])
            nc.sync.dma_start(wqk_sb[:, 8:12], wqk8_d[:, 8:12])
            nc.sync.dma_start(bp_sb[:], bp_d[:, :])
            nc.sync.dma_start(wp_sb[:, :, 0:384], wp_d[:, :, 0:384])
            nc.sync.dma_start(wp_sb[:, :, 384:768], wp_d[:, :, 384:768])
            nc.gpsimd.dma_start(xr_sb[:], xr_d[:, :, :, :])
            nc.gpsimd.dma_start(bv_sb[:], bv_d[:, :])
            nc.gpsimd.dma_start(wv_sb[:, :, 0:384], wv_d[:, :, 0:384])
            nc.gpsimd.dma_start(wv_sb[:, :, 384:768], wv_d[:, :, 384:768])
            nc.gpsimd.dma_start(xT_sb[:, :, 0:512], xT_d[:, :, 0:512])
            nc.gpsimd.dma_start(xT_sb[:, :, 512:1024], xT_d[:, :, 512:1024])
            nc.gpsimd.dma_start(qm_sb[:], qm_d[:, :])
            nc.gpsimd.dma_start(dm_sb[:], dm_d[:, :])
            nc.gpsimd.memset(ones_sb[:], 1.0)
            nc.vector.tensor_copy(onesr_sb[:], ones_sb[:])
            # ones columns interleaved into V (produce softmax sums during PV)
            for t in range(8):
                nc.gpsimd.memset(
                    v_sb[t].rearrange("p (h d) -> p h d", d=65)[:, :, 64:65], 1.0
                )

            # ---------------- phase B helper: one qkT m-tile ----------------
            def emit_qk(m):
                # fp8 DoubleRow, two accumulating groups (x8 then xr)
                for j in range(2):
                    ps = ps_a.tile([128, 512], F32, name="ps", tag="a")
                    for gi, xg in enumerate((x8_sb, xr_sb)):
                        for kt in range(3):
                            nc.tensor.matmul(
                                ps[:],
                                wqk_sb[:, m, kt],
                                xg[:, kt, :, ts(j, 512)],
                                start=(gi == 0 and kt == 0),
                                stop=(gi == 1 and kt == 2),
                                perf_mode=DR,
                            )
                    nc.scalar.activation(
                        qk_sb[m][:, ts(j, 512)],
                        ps[:],
                        AF.Identity,
                        bias=bqk_sb[:, m : m + 1],
                        scale=1.0 / WS,
                    )

            # first head-pair's projections before phase C
            emit_qk(0)
            emit_qk(6)

            # ---------------- phase C: V = x @ W_v + bv ----------------
            for t in range(8):
                for c0, cw in ((0, 512), (512, 256)):
                    psv = ps_a.tile([128, 512], F32, name="psv", tag="a")
                    for k in range(6):
                        nc.tensor.matmul(
                            psv[:, :cw],
                            xT_sb[:, k, ts(t, 128)],
                            wv_sb[:, k, ds(c0, cw)],
                            start=(k == 0),
                            stop=(k == 5),
                        )
                    nh, h0 = cw // 64, c0 // 64
                    nc.vector.tensor_add(
                        v_sb[t].rearrange("p (h d) -> p h d", d=65)[
                            :, h0 : h0 + nh, 0:64
                        ],
                        psv[:, :cw].rearrange("p (h d) -> p h d", d=64),
                        bv_sb[:, ds(c0, cw)].rearrange("p (h d) -> p h d", d=64),
                    )

            # ---------------- phase B+D interleaved per head-pair ----------------
            # kt's processed per head into single-bank [128,512] psums; the
            # normalization block of each (pr, sbi) unit is emitted one unit
            # LATE (norm_q) so its PE broadcast matmul never stalls PE.
            norm_q = []

            def run_norm():
                if norm_q:
                    norm_q.pop(0)()

            for pr in range(6):
                if pr < 5:
                    emit_qk(pr + 1)
                    emit_qk(7 + pr)
                hs = (2 * pr, 2 * pr + 1)
                for sbi in range(2):
                    q0 = sbi * 512
                    nkt = 4 + 4 * sbi
                    psO = {}
                    for h in hs:
                        psO[h] = ps_o.tile([65, 512], F32, name="op", tag="op")
                    pts = {}
                    for kt in range(nkt):
                        dc = max(0, kt * 128 - q0)
                        w = 512 - dc
                        s_psum = {}
                        for h in hs:
                            qp = (h % 2) * 64
                            sp = ps_a.tile([128, 512], F32, name="sp", tag="a")
                            nc.tensor.matmul(
                                sp[:, ds(dc, w)],
                                qk_sb[6 + pr][qp : qp + 64, ts(kt, 128)],
                                qk_sb[pr][qp : qp + 64, ds(q0 + dc, w)],
                                start=True,
                                stop=True,
                            )
                            s_psum[h] = sp
                        for h in hs:
                            ptt = ptp.tile([128, 512], BF16, name="ptt", tag="ptt")
                            nc.scalar.activation(
                                ptt[:, ds(dc, w)],
                                s_psum[h][:, ds(dc, w)],
                                AF.Exp,
                            )
                            if kt * 128 >= q0:
                                nc.vector.tensor_mul(
                                    ptt[:, ds(dc, 128)],
                                    ptt[:, ds(dc, 128)],
                                    dm_sb[:, :],
                                )
                            pts[(h, kt)] = ptt
                        if kt == 1:
                            run_norm()  # previous unit's normalization block
                        for h in hs:
                            nc.tensor.matmul(
                                psO[h][:, ds(dc, w)],
                                v_sb[kt][:, h * 65 : h * 65 + 65],
                                pts[(h, kt)][:, ds(dc, w)],
                                start=(kt == 0),
                                stop=(kt == nkt - 1),
                                skip_group_check=True,
                            )
                    # sums -> sbuf f32r now; broadcast + normalize deferred
                    sums = {}
                    for h in hs:
                        sums[h] = recp.tile(
                            [65, 512], mybir.dt.float32r, name="sums", tag="sums"
                        )
                        nc.vector.tensor_copy(sums[h][64:65, :], psO[h][64:65, :])

                    def mk_norm(pr=pr, sbi=sbi, q0=q0, hs=hs, psO=psO, sums=sums):
                        def go():
                            for h in hs:
                                bc = ps_bc.tile([64, 512], F32, name="bc", tag="bc")
                                nc.tensor.matmul(
                                    bc[:],
                                    onesr_sb[64:65, 0:64],
                                    sums[h][64:65, :],
                                    start=True,
                                    stop=True,
                                )
                                bcs = bcp.tile([64, 512], F32, name="bcs", tag="bcs")
                                nc.vector.reciprocal_approx_fast(bcs[:], bc[:])
                                if h % 2 == 0:
                                    nc.vector.tensor_mul(
                                        ot_sb[0:64, pr, ds(q0, 512)],
                                        psO[h][0:64, :],
                                        bcs[:],
                                    )
                                else:
                                    otx = otxp.tile(
                                        [64, 512], BF16, name="otx", tag="otx"
                                    )
                                    nc.vector.tensor_mul(
                                        otx[:], psO[h][0:64, :], bcs[:]
                                    )
                                    nc.sync.dma_start(
                                        ot_sb[64:128, pr, ds(q0, 512)], otx[:]
                                    )

                        return go

                    norm_q.append(mk_norm())

            # ---------------- phase E: y = OT.T @ W_proj * qm + bp ----------------
            run_norm()  # unit (5, 0)'s block
            run_norm()  # unit (5, 1)'s block
            for qt in range(8):
                ysb = ysbp.tile([128, C], F32, name="ysb", tag="ysb")
                for c0, cw in ((0, 512), (512, 256)):
                    psy = ps_a.tile([128, 512], F32, name="psy", tag="a")
                    for k in range(6):
                        nc.tensor.matmul(
                            psy[:, :cw],
                            ot_sb[:, k, ts(qt, 128)],
                            wp_sb[:, k, ds(c0, cw)],
                            start=(k == 0),
                            stop=(k == 5),
                        )
                    nc.vector.scalar_tensor_tensor(
                        out=ysb[:, ds(c0, cw)],
                        in0=psy[:, :cw],
                        scalar=qm_sb[:, qt : qt + 1],
                        in1=bp_sb[:, ds(c0, cw)],
                        op0=ALU.mult,
                        op1=ALU.add,
                    )
                nc.sync.dma_start(y_d[ts(qt, 128), :], ysb[:])

    nc.compile()
    return nc


def _get_nc():
    if "nc" not in _CACHE:
        _CACHE["nc"] = build_program()
    return _CACHE["nc"]


def _drpack(M, dtype):
    """[768, N] -> [128, 3, 2, N] DoubleRow layout (c = kt*256 + i*128 + p)."""
    N = M.shape[1]
    return np.ascontiguousarray(
        M.reshape(3, 2, 128, N).transpose(2, 0, 1, 3)
    ).astype(dtype)


def _kpack(M, dtype):
    """[768, N] -> [128, 6, N] (row c = k*128 + p)."""
    N = M.shape[1]
    return np.ascontiguousarray(M.reshape(6, 128, N).transpose(1, 0, 2)).astype(
        dtype
    )


def prep_core_inputs(x, mask, query_mask, W_attn, b_attn, W_proj, b_proj):
    """Host-side prep. Returns a list of per-core input dicts."""
    scale = 1.0 / np.sqrt(HD)
    W = np.asarray(W_attn, np.float32)
    Wqk = W[:, : 2 * C].copy()
    Wqk[:, :C] *= scale
    b_s = np.asarray(b_attn, np.float32).copy()
    b_s[:C] *= scale

    wqk8 = _drpack(Wqk * WS, F8NP)
    wqk8 = np.ascontiguousarray(
        wqk8.reshape(128, 3, 2, 12, 128).transpose(0, 3, 1, 2, 4)
    )
    shared = {
        "wqk8": wqk8,
        "wv": _kpack(W[:, 2 * C :], BF16NP),
        "wp": _kpack(np.asarray(W_proj, np.float32), BF16NP),
        "bqk": np.ascontiguousarray(b_s[: 2 * C].reshape(12, 128).T),
        "bv": np.ascontiguousarray(
            np.broadcast_to(b_s[2 * C :], (128, C))
        ).astype(np.float32),
        "bp": np.ascontiguousarray(
            np.broadcast_to(np.asarray(b_proj, np.float32), (128, C))
        ),
        "dm": np.triu(np.ones((128, 128), np.float32)).astype(BF16NP),
    }

    per_core = []
    for b in range(NCORES):
        xT = np.asarray(x[b], np.float32).T
        x8 = _drpack(xT, F8NP)
        xr = (_drpack(xT, np.float32) - x8.astype(np.float32)).astype(F8NP)
        qm = np.ascontiguousarray(
            np.asarray(query_mask[b, 0, :, 0], np.float32).reshape(8, 128).T
        )
        per_core.append(
            {"xT": _kpack(xT, BF16NP), "x8": x8, "xr": xr, "qm": qm, **shared}
        )
    return per_core


def run_on_cores(inputs, trace=False, **kw):
    from concourse.bass_utils import run_bass_kernel_spmd

    nc = _get_nc()
    in_maps = prep_core_inputs(**inputs)
    res = run_bass_kernel_spmd(
        nc, in_maps, core_ids=list(range(NCORES)), trace=trace, **kw
    )
    out = np.stack([res.results[b]["y"] for b in range(NCORES)], axis=0)
    return out.astype(np.float32), res


def kernel(**inputs) -> np.ndarray:
    out, _ = run_on_cores(inputs, trace=False)
    return out
